# revision 1
# baseline (speedup 1.0000x reference)
"""Trainium2 Bass kernel for nn_Attention_34471407518209.

The module computes (all 1x1 convs, BN in training mode):
    q    = Wq2 @ BN(Wq @ x + bq) + bq2
    k    = Wsr @ x + bsr
    attn = rowmax(q @ k^T)            # (B, C, 1)
    out  = Wc @ (attn * mean_c(x))    # outer product against channel-mean

Everything upstream of the rowmax is linear in x, so the whole computation
collapses onto per-batch Gram matrices G_b = x_b x_b^T (64x64), row sums
r_b, and channel means v_b:
    q = A x + c 1^T  with  A = Wq2 diag(g') Wq  (g' from BN stats, which are
    themselves functions of sum_b G_b and sum_b r_b)
    attn_b = [A|c] @ [[G_b, r_b],[r_b^T, N]] @ [Wsr|bsr]^T
    out_b  = (Wc @ rowmax(attn_b)) (x) v_b      # rank-1 outer product

Device phase 1 computes G_b, r_b, v_b (the only pass over x).  x is
marshalled to bf16 on the host (the kernel's Gram/v math is bf16 on the
tensor engine either way - identical numerics to an on-device cast, half
the HBM read traffic).  Per 128-column chunk the x-pair tile is transposed
via a regular matmul against an identity augmented with two batch-mask
columns, so the channel sums v ride along as two extra psum columns; row
sums r ride the gram matmul as a leading all-ones rhs column.  The
transposed v columns are re-oriented to row layout with two small PE
transposes and written out as (2, N) rows per pair.

Host does the tiny 64x64 stats/attn/rowmax math in fp64 between the two
device phases (it is a few microseconds of numpy on 65x65 matrices).

Device phase 2 materializes the (B, C, N) rank-1 outer products
out_b = u_b v_b^T and writes them out.  The host packs u with a K=2
interleaved block-diagonal trick: lhsT[h, 2c+h] = u[c], so ONE matmul
against rhs [v[n0+w]; v[n0+hb/2+w]] (2, 512) fills all 128 psum partitions
with psum[2c+h, w] = u[c] * v[n0 + (hb/2)h + w].  Flattened
partition-major that IS the (c, h, w) element order of out[b, :, n0:n0+hb],
so the staging tile drains with a single plain (128, hb/2) -> (64, hb) DMA
at full 128-partition port bandwidth.

Sharding: data-parallel over batch, 4 batches per core on 8 cores.
"""

import os
from contextlib import ExitStack

import numpy as np
import ml_dtypes

import concourse.bass as bass
import concourse.mybir as mybir
import concourse.tile as tile
from concourse import bacc
from concourse.bass_utils import run_bass_kernel_spmd

B, C, N = 32, 64, 16384
NCORES = 8
BPC = B // NCORES          # batches per core
PAIRS = BPC // 2           # batch pairs per core (2 batches share 128 partitions)
EPS = 1e-5
SLAB = int(os.environ.get("P1_SLAB", "4096"))  # n-columns per DMA slab
CHUNK = 128                # n-columns per transpose/Gram chunk

BF16 = mybir.dt.bfloat16
F32 = mybir.dt.float32
F32R = mybir.dt.float32r

_cache: dict = {}
LAST_RESULTS: dict = {}    # exec-time info for test harnesses


def _run(nc, in_maps, core_ids, trace):
    """run_bass_kernel_spmd with graceful fallback when the axon NTFF
    profiling hook is unavailable (chipless tunnel containers)."""
    if trace:
        try:
            return run_bass_kernel_spmd(nc, in_maps, core_ids, trace=True)
        except ModuleNotFoundError:
            os.environ["BASS_NEVER_TRACE"] = "1"
    return run_bass_kernel_spmd(nc, in_maps, core_ids)


def _build_phase1(rep=None) -> bass.Bass:
    """Per pair: G (128x128 both-batch gram), r (ones column), v (2, N).

    gr[p] layout: col 0 = r, cols 1:129 = G.
    """
    nc = bacc.Bacc(trn_type="TRN2", target_bir_lowering=False)
    x = nc.dram_tensor("x", (PAIRS, 128, N), BF16, kind="ExternalInput")
    consts = nc.dram_tensor("consts", (128, 132), BF16, kind="ExternalInput")
    cf32 = nc.dram_tensor("cf32", (128, 128), F32, kind="ExternalInput")
    gr = nc.dram_tensor("gr", (PAIRS, 128, 129), F32, kind="ExternalOutput")
    vout = nc.dram_tensor("v", (PAIRS, 2, N), F32, kind="ExternalOutput")

    with ExitStack() as ctx:
        tc = ctx.enter_context(tile.TileContext(nc))
        singles = ctx.enter_context(tc.tile_pool(name="singles", bufs=1))
        xpool = ctx.enter_context(tc.tile_pool(
            name="xslab", bufs=int(os.environ.get("P1_XBUFS", "4"))))
        tpool = ctx.enter_context(tc.tile_pool(
            name="tsb", bufs=int(os.environ.get("P1_TBUFS", "12"))))
        tpsum = ctx.enter_context(tc.tile_pool(
            name="tpsum", bufs=int(os.environ.get("P1_TPBUFS", "6")), space="PSUM"))
        grpsum = ctx.enter_context(tc.tile_pool(
            name="grpsum", bufs=int(os.environ.get("P1_GRBUFS", "1")), space="PSUM"))
        vtpsum = ctx.enter_context(tc.tile_pool(name="vtpsum", bufs=1, space="PSUM"))
        vcpool = ctx.enter_context(tc.tile_pool(name="vc", bufs=2))
        opool = ctx.enter_context(tc.tile_pool(name="outs", bufs=2))

        # [I_128 | mask_b0 | mask_b1]: transpose rhs with v ride-along
        ident_aug = singles.tile([128, 130], BF16)
        nc.sync.dma_start(out=ident_aug, in_=consts[:, 0:130])
        ident32 = singles.tile([128, 128], F32)
        nc.sync.dma_start(out=ident32, in_=cf32[:, :])

        # manually-rotated t_sb slots; gram ones-column written once
        TB = int(os.environ.get("P1_TBUFS", "12"))
        t_sbs = [tpool.tile([128, 2, 131], BF16, tag="tsb", name=f"tsb{i}")
                 for i in range(TB)]
        for t in t_sbs:
            nc.vector.memset(t[:, :, 0:1], 1.0)

        n_slabs = N // SLAB
        duos_per_slab = SLAB // (2 * CHUNK)
        n_chunks = N // CHUNK
        # tail outputs ride SWDGE so they never queue behind the bulk
        # x-slab loads in the sync HWDGE FIFO (gpsimd is otherwise idle)
        teng = nc.gpsimd if os.environ.get("P1_TSW", "0") == "1" else nc.sync

        def _body(_iv=None):
          for p in range(PAIRS):
            g_ps = grpsum.tile([128, 129], F32)
            v_cols = vcpool.tile([128, 256], F32, tag="vc")
            for k in range(n_slabs):
                xs = xpool.tile([128, SLAB], BF16)
                if os.environ.get("P1_XALT", "0") == "1":
                    xeng = nc.sync if (p * n_slabs + k) % 2 == 0 else nc.scalar
                else:
                    xeng = nc.sync
                xeng.dma_start(out=xs, in_=x[p, :, k * SLAB:(k + 1) * SLAB])
                for d in range(duos_per_slab):
                    gd = k * duos_per_slab + d       # global duo index 0..63
                    t_ps = tpsum.tile([128, 2, 130], F32)
                    for cc in range(2):
                        c0 = (d * 2 + cc) * CHUNK
                        # regular matmul vs transpose-mode: engages the PE
                        # clock-warmup and FWL; psum cols 128:130 = v cols
                        nc.tensor.matmul(
                            t_ps[:, cc, :],
                            lhsT=xs[:, c0:c0 + CHUNK],
                            rhs=ident_aug,
                            start=True, stop=True)
                    t_sb = t_sbs[(p * n_chunks // 2 + gd) % TB]
                    if gd % 2 == 0:
                        nc.vector.tensor_copy(out=t_sb[:, :, 1:129],
                                              in_=t_ps[:, :, 0:128])
                        nc.scalar.copy(out=v_cols[:, 4 * gd:4 * gd + 4],
                                       in_=t_ps[:, :, 128:130])
                    else:
                        nc.scalar.copy(out=t_sb[:, :, 1:129],
                                       in_=t_ps[:, :, 0:128])
                        nc.vector.tensor_copy(out=v_cols[:, 4 * gd:4 * gd + 4],
                                              in_=t_ps[:, :, 128:130])
                    for cc in range(2):
                        gchunk = gd * 2 + cc
                        nc.tensor.matmul(
                            g_ps,
                            lhsT=t_sb[:, cc, 1:129],
                            rhs=t_sb[:, cc, 0:129],
                            start=(gchunk == 0),
                            stop=(gchunk == n_chunks - 1))
            # v columns (n-part, [2j+b]) -> row layout via two PE transposes
            for blk in range(2):
                vt_ps = vtpsum.tile([128, 128], F32)
                nc.tensor.matmul(
                    vt_ps, lhsT=v_cols[:, 128 * blk:128 * (blk + 1)],
                    rhs=ident32, start=True, stop=True)
                vt_sb = opool.tile([128, 128], F32, tag="vt")
                nc.vector.tensor_copy(out=vt_sb, in_=vt_ps)
                for b in range(2):
                    teng.dma_start(
                        out=vout[p, b, 8192 * blk:8192 * (blk + 1)].rearrange(
                            "(j w) -> j w", j=64),
                        in_=vt_sb[b::2, :])
            gr_sb = opool.tile([128, 129], F32, tag="grsb")
            nc.vector.tensor_copy(out=gr_sb, in_=g_ps)
            teng.dma_start(out=gr[p], in_=gr_sb)

        if rep is None:
            _body()
        else:
            with tc.For_i(0, rep, 1) as _iv:
                _body(_iv)

    nc.compile()
    return nc


def _build_phase2(rep=None) -> bass.Bass:
    nc = bacc.Bacc(trn_type="TRN2", target_bir_lowering=False)
    u = nc.dram_tensor("u", (2, BPC * 128), F32R, kind="ExternalInput")
    v = nc.dram_tensor("v", (BPC, N), F32R, kind="ExternalInput")
    out = nc.dram_tensor("out", (BPC, C, N), F32, kind="ExternalOutput")

    with ExitStack() as ctx:
        tc = ctx.enter_context(tile.TileContext(nc))
        upool = ctx.enter_context(tc.tile_pool(name="u", bufs=1))
        vpool = ctx.enter_context(tc.tile_pool(
            name="v", bufs=int(os.environ.get("P2_VBUFS", "3"))))
        opsum = ctx.enter_context(tc.tile_pool(name="opsum", bufs=8, space="PSUM"))
        obuf = ctx.enter_context(tc.tile_pool(
            name="obuf", bufs=int(os.environ.get("P2_OBUFS", "3"))))

        u_t = upool.tile([2, BPC * 128], F32R)
        nc.sync.dma_start(out=u_t, in_=u[:, :])

        HB = int(os.environ.get("P2_HB", "16384"))  # n-range per (v_t, ob) pair

        P2_ALT = os.environ.get("P2_ALT", "1") == "1"

        # v loads ride SWDGE so they never queue behind the 4MB output
        # writes in an HWDGE FIFO (HWDGE is FIFO per issuing engine)
        veng = nc.gpsimd if os.environ.get("P2_VSW", "1") == "1" else nc.sync

        def _group(b, u_b, n0, hb, gi):
            v_t = vpool.tile([2, hb // 2], F32R, tag="v_t")
            veng.dma_start(
                out=v_t,
                in_=v[b:b + 1, n0:n0 + hb].rearrange(
                    "one (r w) -> (one r) w", r=2),
            )
            ob = obuf.tile([128, hb // 2], F32, tag="ob")
            for t in range(hb // 1024):
                o_ps = opsum.tile([128, 512], F32)
                nc.tensor.matmul(
                    o_ps, lhsT=u_b,
                    rhs=v_t[:, t * 512:(t + 1) * 512],
                    start=True, stop=True,
                )
                if t % 2 == 0:
                    nc.vector.tensor_copy(out=ob[:, t * 512:(t + 1) * 512], in_=o_ps)
                else:
                    nc.scalar.copy(out=ob[:, t * 512:(t + 1) * 512], in_=o_ps)
            nways = int(os.environ.get("P2_NWAYS", "2")) if P2_ALT else 1
            deng = [nc.sync, nc.scalar, nc.gpsimd][gi % nways]
            deng.dma_start(out=out[b, :, n0:n0 + hb], in_=ob)

        def _body(_iv=None):
          gi = 0
          for b in range(BPC):
            u_b = u_t[:, b * 128:(b + 1) * 128]
            n0 = 0
            hbs = [HB] * (N // HB)
            if b == BPC - 1 and os.environ.get("P2_TELE", "0") == "1":
                # shrink the final (un-overlapped) writes
                hbs = hbs[:-1] + [HB // 2, HB // 4, HB // 8, HB // 8]
            if b == 0 and os.environ.get("P2_RAMP", "1") == "1":
                # shrink the first writes so the output DMA starts early
                hbs = [HB // 8, HB // 8, HB // 4, HB // 2] + hbs[1:]
            for hb in hbs:
                _group(b, u_b, n0, hb, gi)
                n0 += hb
                gi += 1

        if rep is None:
            _body()
        else:
            with tc.For_i(0, rep, 1) as _iv:
                _body(_iv)

    nc.compile()
    return nc


def _consts_np() -> np.ndarray:
    consts = np.zeros((128, 132), dtype=ml_dtypes.bfloat16)
    consts[:, 0:128] = np.eye(128, dtype=np.float32).astype(ml_dtypes.bfloat16)
    consts[0:64, 128] = 1.0      # mask col: batch-even channels
    consts[64:128, 129] = 1.0    # mask col: batch-odd channels
    return consts


def _host_math(G, r, Wq, bq, gamma, beta, Wq2, bq2, Wsr, bsr, Wc):
    """G: (B, C, C), r: (B, C) in fp64. Returns u: (B, C) fp64."""
    M = G.sum(axis=0) / (B * N)
    m = r.sum(axis=0) / (B * N)
    mu = Wq @ m + bq
    Eq2 = np.einsum("ij,jk,ik->i", Wq, M, Wq) + 2 * bq * (Wq @ m) + bq * bq
    var = Eq2 - mu * mu
    gp = gamma / np.sqrt(var + EPS)
    betap = beta - mu * gp
    A = Wq2 @ (gp[:, None] * Wq)
    c = Wq2 @ (gp * bq + betap) + bq2

    Aa = np.concatenate([A, c[:, None]], axis=1)            # (C, C+1)
    Wa = np.concatenate([Wsr, bsr[:, None]], axis=1)        # (C, C+1)
    u = np.zeros((B, C))
    for b in range(B):
        Ga = np.zeros((C + 1, C + 1))
        Ga[:C, :C] = G[b]
        Ga[:C, C] = r[b]
        Ga[C, :C] = r[b]
        Ga[C, C] = N
        attn = Aa @ Ga @ Wa.T
        u[b] = Wc @ attn.max(axis=1)
    return u


def kernel(x, Wq, bq, gamma, beta, Wq2, bq2, Wsr, bsr, Wc, H=None, W=None, **_):
    x = np.asarray(x)
    Wq = np.asarray(Wq, dtype=np.float64)
    bq = np.asarray(bq, dtype=np.float64)
    gamma = np.asarray(gamma, dtype=np.float64)
    beta = np.asarray(beta, dtype=np.float64)
    Wq2 = np.asarray(Wq2, dtype=np.float64)
    bq2 = np.asarray(bq2, dtype=np.float64)
    Wsr = np.asarray(Wsr, dtype=np.float64)
    bsr = np.asarray(bsr, dtype=np.float64)
    Wc = np.asarray(Wc, dtype=np.float64)

    if "p1" not in _cache:
        _cache["p1"] = _build_phase1()
        _cache["p2"] = _build_phase2()
    nc1, nc2 = _cache["p1"], _cache["p2"]

    trace = bool(os.environ.get("BASS_TRACE"))
    consts = _consts_np()
    cf32 = np.eye(128, dtype=np.float32)
    core_ids = list(range(NCORES))

    # host-side input marshalling: shard over batch, bf16 (the device Gram
    # math is bf16 regardless; this halves the phase-1 HBM read)
    xb = np.ascontiguousarray(
        np.asarray(x, dtype=np.float32)).astype(ml_dtypes.bfloat16)
    in_maps1 = []
    for i in range(NCORES):
        xc = xb[BPC * i: BPC * (i + 1)].reshape(PAIRS, 128, N)
        in_maps1.append({"x": xc, "consts": consts, "cf32": cf32})
    res1 = _run(nc1, in_maps1, core_ids, trace)
    LAST_RESULTS["p1"] = res1

    # unpack per-core results
    G = np.zeros((B, C, C))
    r = np.zeros((B, C))
    v = np.zeros((B, N), dtype=np.float32)
    for i in range(NCORES):
        gr_i = np.asarray(res1.results[i]["gr"], dtype=np.float64)
        v_i = np.asarray(res1.results[i]["v"])
        for p in range(PAIRS):
            b0 = BPC * i + 2 * p
            G[b0] = gr_i[p, 0:64, 1:65]
            G[b0 + 1] = gr_i[p, 64:128, 65:129]
            r[b0] = gr_i[p, 0:64, 0]
            r[b0 + 1] = gr_i[p, 64:128, 0]
            v[b0:b0 + 2] = v_i[p]

    u = _host_math(G, r, Wq, bq, gamma, beta, Wq2, bq2, Wsr, bsr, Wc)
    # device v is the channel *sum*; the reference uses the channel mean.
    u = np.ascontiguousarray(u / C, dtype=np.float32)

    in_maps2 = []
    for i in range(NCORES):
        uc = u[BPC * i: BPC * (i + 1)]              # (BPC, 64)
        u2 = np.zeros((2, BPC * 128), dtype=np.float32)
        for b in range(BPC):
            u2[0, b * 128: (b + 1) * 128: 2] = uc[b]   # lhsT[0, 2c]   = u[c]
            u2[1, b * 128 + 1: (b + 1) * 128: 2] = uc[b]  # lhsT[1, 2c+1] = u[c]
        in_maps2.append({
            "u": u2,
            "v": np.ascontiguousarray(v[BPC * i: BPC * (i + 1)]),
        })
    res2 = _run(nc2, in_maps2, core_ids, trace)
    LAST_RESULTS["p2"] = res2

    out = np.empty((B, C, N), dtype=np.float32)
    for i in range(NCORES):
        out[BPC * i: BPC * (i + 1)] = res2.results[i]["out"]
    return out

